# revision 18
# baseline (speedup 1.0000x reference)
"""OFA attention (dense_transformer) on 8 Trainium2 NeuronCores — v2.6.

Sharding: heads split over cores (core c owns heads {2c, 2c+1}, both batches).
Measured: ~306us HW exec, rel err 5.6e-3 (vs 384-406us baseline).

Per-core program (three phases, one nc, deep software pipeline):
  A (head, ~15us): consts + hsT-chunk0 DMA + K0/Q0 projections only; the ebs
     (exp-bias) tiles for the first t-block are interleaved with the remaining
     hsT chunks on the sync queue.  A dummy exp preloads the ACT table.
  B: attention b0.  Per t-block, 16 s-tile steps (2 heads each):
       scores -> st PSUM f32 [128,2,512] (plane = head), pool bufs=2 so the
                 next scores run DURING the current exp (no WAR serialization)
       exp    -> ScalarE ACTIVATE FD=1024 into an e_r pair tile; ScalarE does
                 ONLY exp + wo drains: exp(s+b) == exp(s)*exp(b), bias exp'd
                 on host (bf16) and DMA'd as contiguous 1MB per sp-pair
       mult   -> e_t = e_r * ebs, one DVE op per sp-PAIR (FD=4096, bf16 2x)
       PV     -> [V|1].T @ E accumulates O.T + softmax sums in PSUM f32;
                 groups queued >=4 deep so the PE always has ready work
     K1-3/Q1-3/V-b0 projections run as tasks popped under tb0 (V in NATURAL
     orientation: stationary = hsT chunk -> no VT/transpose phase; biases
     folded in via ones-row matmuls so drains are pure copies).
     b1's projections pop 1 task/sp under tb1-tb3, gated on their hsT DMAs.
     Block drains spread over the NEXT block as small tasks (no boundary
     bunching -> no PE idle window):
       O.T+sums -> SBUF bf16; PE-transpose puts tokens into PARTITIONS, so
       1/sums applies as a per-partition DVE scalar; PE-transpose back and the
       out-projection runs heads-FUSED (contraction 128) with ScalarE
       pure-copy drains.  No DRAM sums-roundtrip.
  C: attention b1 (same pipeline).  The LAST block's drain tasks round-robin
     across all three PSUM pools so the tail chains run in parallel.
Host: partial outputs summed over cores + bo (the all-reduce of out_proj).

PSUM budget (8 banks): st 2x2 + ot 2 + aux 2 (one rotating bank-sized tag
shared by fwd/back transposes, wo matmuls and task-popped proj groups).

Notes from failed experiments (do not retry blindly):
  - fp8e4m3 for E (probs) via DoubleRow PV: rel err 2.4e-2 (subnormal flush +
    6% mantissa steps on probs) AND slower (the gpsimd bf16->fp8 cast
    serialized the pipeline).
  - fp8e4m3 for Q/K/V projections (x16 weight scaling): rel err 4.5e-2 —
    score noise amplifies through exp ~5x worse than a naive error model.
  - PE runs mostly at the cold 1.2GHz HAM clock (MM avg ~370-390ns for N=512)
    despite ~80% busy; micro-gaps keep re-throttling it (K18-style).  PE is
    the binding engine at ~247us busy; ScalarE ~176, DVE ~145.
"""
import sys
from collections import deque

for _p in ("/opt/trn_rl_repo",):
    if _p not in sys.path:
        sys.path.append(_p)

import numpy as np

import concourse.bass as bass
import concourse.tile as tile
from concourse import mybir
from concourse.masks import make_identity
from concourse.bass_utils import run_bass_kernel_spmd

F32 = mybir.dt.float32
BF16 = mybir.dt.bfloat16
F8 = mybir.dt.float8e4

B, T, E, NH, D = 2, 2048, 1024, 16, 64
N_CORES = 8
HPC = NH // N_CORES          # 2 heads per core
DH = HPC * D                 # 128
SCALING = float(D * 2.0) ** -0.5
T_BLOCK = 512
NTB = T // T_BLOCK           # 4 t-blocks per batch
NSP = 8                      # sp-steps per block (2 s-tiles each)
NST = 16                     # s-tiles per batch
NE = E // 128                # 8 e-strips
TOK = B * T


def _waitfix(nc, limit=1):
    """This walrus build accepts at most ONE sync-wait per instruction.
    Hoist excess sem-waits onto inserted single-wait NoOps."""
    n_fixed = 0
    for bb in nc.m.functions[0].blocks:
        i = 0
        insts = bb.instructions
        while i < len(insts):
            inst = insts[i]
            si = inst.sync_info
            if si and si.on_wait and len(si.on_wait) > limit:
                extra = si.on_wait[limit:]
                si.on_wait = si.on_wait[:limit]
                for k, w in enumerate(extra):
                    nop = mybir.InstNoOp(
                        name=f"{inst.name}-waitfix{k}",
                        engine=inst.engine,
                        sync_info=mybir.SyncInfo(on_wait=[w], on_update=[]),
                        bass_nofuse=True,
                    )
                    nc.register_instruction(nop, overwrite=True)
                    insts.insert(i, nop)
                    i += 1
                n_fixed += 1
            i += 1
    return n_fixed


def build_attention_nc():
    nc = bass.Bass()

    hsT = nc.declare_dram_parameter("hsT", [E, TOK], BF16, isOutput=False)
    wqT = nc.declare_dram_parameter("wqT", [E, DH], BF16, isOutput=False)
    wkT = nc.declare_dram_parameter("wkT", [E, DH], BF16, isOutput=False)
    wvT = nc.declare_dram_parameter("wvT", [E, DH], BF16, isOutput=False)
    brows = nc.declare_dram_parameter("brows", [1, 3 * DH], BF16, isOutput=False)
    woT = nc.declare_dram_parameter("woT", [DH, E], BF16, isOutput=False)
    # exp(bias+mask), pre-arranged on host to per-(block, sp-pair) tiles:
    # [B, NTB, 4 pairs, 128 p, 2 spi, 2 a, 2 half, 512 t]
    bias_in = nc.declare_dram_parameter("bias",
                                        [B, NTB, 4, 128, 2, 2, 2, T_BLOCK],
                                        BF16, isOutput=False)
    out_partial = nc.declare_dram_parameter("out", [TOK, E], BF16, isOutput=True)

    with tile.TileContext(nc) as tc:
        from contextlib import ExitStack
        with ExitStack() as ctx:
            consts = ctx.enter_context(tc.tile_pool(name="consts", bufs=1))
            persist = ctx.enter_context(tc.tile_pool(name="persist", bufs=1))
            ebs_pool = ctx.enter_context(tc.tile_pool(name="ebs", bufs=4))
            er_pool = ctx.enter_context(tc.tile_pool(name="er", bufs=2))
            et_pool = ctx.enter_context(tc.tile_pool(name="et", bufs=2))
            hst_pool = ctx.enter_context(tc.tile_pool(name="hst", bufs=64))
            otn65_pool = ctx.enter_context(tc.tile_pool(name="otn65", bufs=2))
            onat_pool = ctx.enter_context(tc.tile_pool(name="onat", bufs=2))
            otnn_pool = ctx.enter_context(tc.tile_pool(name="otnn", bufs=2))
            rsb_pool = ctx.enter_context(tc.tile_pool(name="rsb", bufs=2))
            os_pool = ctx.enter_context(tc.tile_pool(name="osb", bufs=2))

            i_bf = consts.tile([128, 128], BF16, tag="i_bf")
            make_identity(nc, i_bf[:])
            ones_row = consts.tile([1, T_BLOCK], BF16, tag="ones_row")
            nc.gpsimd.memset(ones_row[:], 1.0)

            # dummy exp: pull the ACT exp table-load off the critical path
            dummy = consts.tile([128, 1], BF16, tag="dummy")
            nc.scalar.activation(out=dummy[:], in_=i_bf[:, 0:1],
                                 func=mybir.ActivationFunctionType.Exp)

            # weights: (E, DH) -> (128, NE, DH) bf16
            w_sb = {}
            for name, src in (("wq", wqT), ("wk", wkT), ("wv", wvT)):
                t = consts.tile([128, NE, DH], BF16, tag=name)
                nc.sync.dma_start(out=t[:],
                                  in_=src.rearrange("(n p) d -> p n d", p=128))
                w_sb[name] = t
            wo_sb = consts.tile([128, E], BF16, tag="wo")
            nc.sync.dma_start(out=wo_sb[:], in_=woT[:, :])
            b_rows = consts.tile([1, 3 * DH], BF16, tag="b_rows")
            nc.sync.dma_start(out=b_rows[:], in_=brows[:, :])

            # persistent activations
            QTb = [persist.tile([128, T], BF16, tag=f"QT{bb}", name=f"QT{bb}")
                   for bb in range(B)]
            KTb = [persist.tile([128, T], BF16, tag=f"KT{bb}", name=f"KT{bb}")
                   for bb in range(B)]
            V_sbb = []
            for bb in range(B):
                V_sb = persist.tile([128, NST, 256], BF16, tag=f"V_sb{bb}",
                                    name=f"V_sb{bb}")
                nc.vector.memset(V_sb[:, :, :], 0.0)
                nc.vector.memset(V_sb[:, :, D:D + 1], 1.0)
                nc.vector.memset(V_sb[:, :, 128 + D:128 + D + 1], 1.0)
                V_sbb.append(V_sb)

            # ---------- hsT strip DMA (per batch, chunk-major) ----------
            hstrips = {}

            def emit_hst_dma(bb, c, e):
                h = hst_pool.tile([128, T_BLOCK], BF16, tag="hst",
                                  name=f"hst{bb}_{c}_{e}")
                nc.sync.dma_start(
                    out=h[:], in_=hsT[e * 128:(e + 1) * 128,
                                      bb * T + c * T_BLOCK:
                                      bb * T + (c + 1) * T_BLOCK])
                hstrips[(bb, c, e)] = h

            # ---------- projection emitters (psum_pool passed in) ----------
            def emit_qk_group(pool, bb, name, dst, c):
                ps = pool.tile([128, T_BLOCK], F32, tag="aux",
                               name=f"pqk{bb}_{name}_{c}")
                for e in range(NE):
                    nc.tensor.matmul(ps[:], w_sb[name][:, e, :],
                                     hstrips[(bb, c, e)][:],
                                     start=(e == 0), stop=False)
                bi = {"wq": 0, "wk": 1}[name]
                nc.tensor.matmul(ps[:], b_rows[0:1, bi * DH:(bi + 1) * DH],
                                 ones_row[:], start=False, stop=True)
                nc.vector.tensor_copy(
                    out=dst[:, c * T_BLOCK:(c + 1) * T_BLOCK], in_=ps[:])

            def emit_v_group(pool, bb, st):
                # V natural: out [128 t, 128 dh] = sum_e hsT_chunk.T @ wvT
                c, q = st // 4, st % 4        # chunk, 128-col quarter
                ps = pool.tile([128, T_BLOCK], F32, tag="aux",
                               name=f"pv{bb}_{st}")
                for e in range(NE):
                    nc.tensor.matmul(
                        ps[:, 0:DH],
                        hstrips[(bb, c, e)][:, q * 128:(q + 1) * 128],
                        w_sb["wv"][:, e, :],
                        start=(e == 0), stop=False)
                nc.tensor.matmul(ps[:, 0:DH], ones_row[:, 0:128],
                                 b_rows[0:1, 2 * DH:3 * DH],
                                 start=False, stop=True)
                nc.vector.tensor_copy(out=V_sbb[bb][:, st, 0:D],
                                      in_=ps[:, 0:D])
                nc.vector.tensor_copy(out=V_sbb[bb][:, st, 128:128 + D],
                                      in_=ps[:, D:DH])

            # ---------- phase A: b0 K0/Q0 only (rest under tb0) ----------
            with tc.tile_pool(name="proj0", bufs=3, space="PSUM") as proj0:
                for e in range(NE):
                    emit_hst_dma(0, 0, e)
                emit_qk_group(proj0, 0, "wk", KTb[0], 0)
                emit_qk_group(proj0, 0, "wq", QTb[0], 0)

            # ---------- phases B & C: attention (+ b1 proj under B) ----------
            with tc.tile_pool(name="st_ps", bufs=2, space="PSUM") as st_ps, \
                 tc.tile_pool(name="ot_ps", bufs=2, space="PSUM") as ot_ps, \
                 tc.tile_pool(name="aux_ps", bufs=2, space="PSUM") as aux_ps:

                tasks = deque()       # small closures: drains of prev block
                proj_tasks = deque()  # b1 projection closures (phase B)
                pend_pv = deque()     # queued PV groups

                def emit_pv_group(group):
                    ots_p, bb_, e_t_t, spi_, sp_ = group
                    for a in range(HPC):
                        for half in range(2):
                            pst = sp_ * 2 + half
                            nc.tensor.matmul(
                                ots_p[a][:],
                                V_sbb[bb_][:, pst, a * 128:a * 128 + 128],
                                e_t_t[:, spi_, a, half, :],
                                start=(pst == 0), stop=(pst == NST - 1))

                def enqueue_block_tasks(b_, tb_, ots_, pools=None):
                    pools = pools or [aux_ps]
                    pcyc = [0]

                    def psum_tile(shape, dtype, nm):
                        pool = pools[pcyc[0] % len(pools)]
                        pcyc[0] += 1
                        tg = {id(aux_ps): "aux", id(st_ps): "st",
                              id(ot_ps): "ot"}[id(pool)]
                        return pool.tile(shape, dtype, tag=tg, name=nm)
                    tglob = b_ * T + tb_ * T_BLOCK
                    otn65 = otn65_pool.tile([128, HPC, T_BLOCK], BF16,
                                            tag="otn65", name=f"otn65_{b_}_{tb_}")
                    o_natn = onat_pool.tile([128, 8, D], BF16, tag="onat",
                                            name=f"onat_{b_}_{tb_}")
                    otn_n = otnn_pool.tile([128, T_BLOCK], BF16, tag="otnn",
                                           name=f"otnn_{b_}_{tb_}")
                    r_sb = rsb_pool.tile([128, 8], F32, tag="rsb",
                                         name=f"rsb_{b_}_{tb_}")
                    os_t = os_pool.tile([128, 4, E], BF16, tag="osb",
                                        name=f"osb_{b_}_{tb_}")

                    # otn65 copies emitted inline (all PV of this block already
                    # emitted) so later pool reuse sees the readers in order.
                    for a in range(HPC):
                        nc.vector.tensor_copy(out=otn65[0:D + 1, a, :],
                                              in_=ots_[a][0:D + 1, :])

                    def t_fwd(k, a):
                        idx = k * 2 + a
                        tr = psum_tile([128, D + 1], BF16,
                                       f"trf{b_}_{tb_}_{k}_{a}")
                        nc.tensor.transpose(
                            tr[:, :],
                            otn65[0:D + 1, a, k * 128:(k + 1) * 128],
                            i_bf[0:D + 1, 0:D + 1])
                        nc.vector.reciprocal(r_sb[:, idx:idx + 1],
                                             tr[:, D:D + 1])
                        nc.vector.tensor_scalar(
                            out=o_natn[:, idx, :], in0=tr[:, 0:D],
                            scalar1=r_sb[:, idx:idx + 1], scalar2=None,
                            op0=mybir.AluOpType.mult)

                    def t_back(k):
                        trb = psum_tile([128, 128], BF16,
                                        f"trb{b_}_{tb_}_{k}")
                        # [128 t, (2 a, 64 d)] -> [(a,d)=128, 128 t] in one shot
                        nc.tensor.transpose(trb[:, :],
                                            o_natn[:, k * 2:k * 2 + 2, :],
                                            i_bf[:, :])
                        nc.vector.tensor_copy(
                            out=otn_n[:, k * 128:(k + 1) * 128], in_=trb[:, :])

                    def t_wo(k, n0):
                        wp = psum_tile([128, 512], F32,
                                       f"wo{b_}_{tb_}_{k}_{n0}")
                        nc.tensor.matmul(wp[:], otn_n[:, k * 128:(k + 1) * 128],
                                         wo_sb[:, n0:n0 + 512],
                                         start=True, stop=True)
                        nc.scalar.activation(
                            out=os_t[:, k, n0:n0 + 512], in_=wp[:],
                            func=mybir.ActivationFunctionType.Copy)

                    def t_dma(k):
                        nc.gpsimd.dma_start(
                            out=out_partial[tglob + k * 128:
                                            tglob + (k + 1) * 128, :],
                            in_=os_t[:, k, :])

                    for k in range(4):
                        tasks.append(lambda k=k: t_fwd(k, 0))
                        tasks.append(lambda k=k: t_fwd(k, 1))
                        tasks.append(lambda k=k: t_back(k))
                        tasks.append(lambda k=k: t_wo(k, 0))
                        tasks.append(lambda k=k: t_wo(k, 512))
                        tasks.append(lambda k=k: t_dma(k))

                # b1 proj tasks (popped under b0 attention, phase B);
                # each entry = (#strips it needs emitted, closure)
                def enqueue_b1_proj():
                    for c in range(4):
                        need = (c + 1) * NE
                        proj_tasks.append((need,
                            lambda c=c: emit_qk_group(aux_ps, 1, "wk", KTb[1], c)))
                        proj_tasks.append((need,
                            lambda c=c: emit_qk_group(aux_ps, 1, "wq", QTb[1], c)))
                        for st in range(c * 4, c * 4 + 4):
                            proj_tasks.append((need,
                                lambda st=st: emit_v_group(aux_ps, 1, st)))
                enqueue_b1_proj()

                b0_tasks = deque()
                for c in range(1, 4):
                    b0_tasks.append(
                        lambda c=c: emit_qk_group(aux_ps, 0, "wk", KTb[0], c))
                for st in range(NST):
                    b0_tasks.append(lambda st=st: emit_v_group(aux_ps, 0, st))
                for c in range(1, 4):
                    b0_tasks.append(
                        lambda c=c: emit_qk_group(aux_ps, 0, "wq", QTb[0], c))

                nb1_strips = [0]

                def emit_b1_strips(n):
                    while nb1_strips[0] < 32 and n > 0:
                        i = nb1_strips[0]
                        emit_hst_dma(1, i // NE, i % NE)
                        nb1_strips[0] += 1
                        n -= 1

                ebs_tiles = {}

                def prefetch_ebs(b_, tb_, pair_):
                    if tb_ >= NTB:
                        b_, tb_ = b_ + 1, tb_ - NTB
                    if b_ >= B or (b_, tb_, pair_) in ebs_tiles:
                        return
                    t = ebs_pool.tile([128, 2, 2, 2, T_BLOCK], BF16, tag="ebs",
                                      name=f"ebs{b_}_{tb_}_{pair_}")
                    nc.sync.dma_start(out=t[:], in_=bias_in[b_, tb_, pair_])
                    ebs_tiles[(b_, tb_, pair_)] = t

                for pr in range(3):
                    prefetch_ebs(0, 0, pr)
                    for e in range(NE):
                        emit_hst_dma(0, pr + 1, e)
                prefetch_ebs(0, 0, 3)

                for b in range(B):
                    for tb in range(NTB):
                        ots = [ot_ps.tile([128, T_BLOCK], F32, tag="ot",
                                          name=f"ot{b}_{tb}_{a}")
                               for a in range(HPC)]
                        for pair in range(4):
                            prefetch_ebs(b, tb + (pair + 4) // 4,
                                         (pair + 4) % 4)
                            if b == 0:
                                emit_b1_strips(4)
                            ebs_t = ebs_tiles.pop((b, tb, pair))
                            e_r = er_pool.tile([128, 2, 2, 2, T_BLOCK], BF16,
                                               tag="er", name=f"er{b}_{tb}_{pair}")
                            e_t = et_pool.tile([128, 2, 2, 2, T_BLOCK], BF16,
                                               tag="et", name=f"et{b}_{tb}_{pair}")
                            for spi in range(2):
                                sp = pair * 2 + spi
                                # side work first: PE chews ready ops while
                                # the next scores' WAR resolves
                                if b == 0:
                                    for _ in range(3):
                                        if b0_tasks:
                                            b0_tasks.popleft()()
                                if (b == 0 and tb >= 1 and proj_tasks
                                        and nb1_strips[0] >= proj_tasks[0][0]):
                                    proj_tasks.popleft()[1]()
                                for _ in range(4):
                                    if tasks:
                                        tasks.popleft()()
                                for half in range(2):
                                    sti = sp * 2 + half
                                    st_t = st_ps.tile([128, 2, T_BLOCK], F32,
                                                      tag="st",
                                                      name=f"st{b}_{tb}_{sti}")
                                    for a in range(HPC):
                                        r0 = a * D
                                        for m0 in (0, 64):
                                            nc.tensor.matmul(
                                                st_t[m0:m0 + 64, a, :],
                                                KTb[b][r0:r0 + D,
                                                       sti * 128 + m0:
                                                       sti * 128 + m0 + 64],
                                                QTb[b][r0:r0 + D,
                                                       tb * T_BLOCK:
                                                       (tb + 1) * T_BLOCK],
                                                start=True, stop=True)
                                    nc.scalar.activation(
                                        out=e_r[:, spi, :, half, :],
                                        in_=st_t[:],
                                        func=mybir.ActivationFunctionType.Exp)
                            nc.vector.tensor_mul(out=e_t[:], in0=e_r[:],
                                                 in1=ebs_t[:])
                            for spi in range(2):
                                sp = pair * 2 + spi
                                pend_pv.append((ots, b, e_t, spi, sp))
                            while len(pend_pv) > 4:
                                emit_pv_group(pend_pv.popleft())
                        # flush this block's remaining PV groups before the
                        # drain tasks reference ots (emission-order = dep-order)
                        while pend_pv:
                            emit_pv_group(pend_pv.popleft())
                        last = (b == B - 1 and tb == NTB - 1)
                        enqueue_block_tasks(
                            b, tb, ots,
                            pools=[aux_ps, st_ps, ot_ps] if last else None)
                # epilogue
                while b0_tasks:
                    b0_tasks.popleft()()
                while proj_tasks:
                    proj_tasks.popleft()[1]()
                while tasks:
                    tasks.popleft()()
    _waitfix(nc)
    return nc


# ---------------- host-side prep ----------------

def shard_inputs(hidden_states, attn_bias, attention_mask, Wq, bq, Wk, bk, Wv, bv,
                 Wo, bo, c_attn, n_cores=8, scaling=None):
    """Build per-core input maps. Returns (in_maps, with_mask)."""
    import ml_dtypes
    bf16 = ml_dtypes.bfloat16
    Bb, Tt, Ee = hidden_states.shape
    NHh = c_attn.shape[0]
    Dd = Ee // NHh
    HPCc = NHh // n_cores
    DHh = HPCc * Dd

    with_mask = bool(np.any(attention_mask))
    hsT = np.ascontiguousarray(hidden_states.reshape(Bb * Tt, Ee).T).astype(bf16)
    bias4 = attn_bias.reshape(Bb, NHh, Tt, Tt)
    if with_mask:
        bias4 = bias4 + attention_mask.reshape(Bb, 1, Tt, Tt)

    if scaling is None:
        scaling = float(Dd * 2.0) ** -0.5

    expb_u16 = np.exp(bias4).astype(bf16).view(np.uint16)

    in_maps = []
    for c in range(n_cores):
        r0 = c * DHh
        sl = slice(r0, r0 + DHh)
        hsl = slice(c * HPCc, (c + 1) * HPCc)
        cvec = np.repeat(c_attn[c * HPCc:(c + 1) * HPCc], Dd)
        # ebs: [B, NTB, 4 pair, 128 p, 2 spi, 4 plane(2*half+a), 512 t]
        # value = exp(bias)[b, a, t, s] with s = (((pair*2+spi)*2+half)*128+p)
        eb = expb_u16[:, hsl]                       # [B, 2, T(t), S(s)]
        eb = eb.reshape(Bb, HPCc, NTB, T_BLOCK, 4, 2, 2, 128)
        # axes: b, a, tb, t', pair, spi, half, p -> b, tb, pair, p, spi, a, half, t'
        ebs = np.ascontiguousarray(eb.transpose(0, 2, 4, 7, 5, 1, 6, 3))
        brows = np.concatenate([(bq[sl] * scaling), bk[sl],
                                  (bv[sl] * cvec)])[None, :].astype(bf16)
        m = {
            "hsT": hsT,
            "wqT": np.ascontiguousarray((Wq[sl] * scaling).T).astype(bf16),
            "wkT": np.ascontiguousarray(Wk[sl].T).astype(bf16),
            "wvT": np.ascontiguousarray((Wv[sl] * cvec[:, None]).T).astype(bf16),
            "brows": brows,
            "woT": np.ascontiguousarray(Wo[:, sl].T).astype(bf16),
            "bias": ebs.view(bf16),
        }
        in_maps.append(m)
    return in_maps, with_mask


_NC_CACHE = {}


def run_spmd(in_maps, with_mask=False, **kwargs):
    if "nc" not in _NC_CACHE:
        _NC_CACHE["nc"] = build_attention_nc()
    nc = _NC_CACHE["nc"]
    return run_bass_kernel_spmd(nc, in_maps, list(range(N_CORES)), **kwargs)


def kernel(hidden_states, attn_bias, attention_mask, Wq, bq, Wk, bk, Wv, bv,
           Wo, bo, c_attn):
    args = [np.asarray(a, dtype=np.float32) for a in
            (hidden_states, attn_bias, attention_mask, Wq, bq, Wk, bk, Wv, bv,
             Wo, bo, c_attn)]
    (hidden_states, attn_bias, attention_mask, Wq, bq, Wk, bk, Wv, bv,
     Wo, bo, c_attn) = args
    in_maps, with_mask = shard_inputs(hidden_states, attn_bias, attention_mask,
                                      Wq, bq, Wk, bk, Wv, bv, Wo, bo, c_attn,
                                      n_cores=N_CORES, scaling=SCALING)
    res = run_spmd(in_maps, with_mask)
    out = np.zeros((B * T, E), np.float32)
    for r in res.results:
        out += r["out"]
    out += bo[None, :]
    return out.reshape(B, T, E).astype(np.float32)


# revision 19
# speedup vs baseline: 1.0466x; 1.0466x over previous
"""OFA attention (dense_transformer) on 8 Trainium2 NeuronCores — v2.6.

Sharding: heads split over cores (core c owns heads {2c, 2c+1}, both batches).
Measured: ~306us HW exec, rel err 5.6e-3 (vs 384-406us baseline).

Per-core program (three phases, one nc, deep software pipeline):
  A (head, ~15us): consts + hsT-chunk0 DMA + K0/Q0 projections only; the ebs
     (exp-bias) tiles for the first t-block are interleaved with the remaining
     hsT chunks on the sync queue.  A dummy exp preloads the ACT table.
  B: attention b0.  Per t-block, 16 s-tile steps (2 heads each):
       scores -> st PSUM f32 [128,2,512] (plane = head), pool bufs=2 so the
                 next scores run DURING the current exp (no WAR serialization)
       exp    -> ScalarE ACTIVATE FD=1024 into an e_r pair tile; ScalarE does
                 ONLY exp + wo drains: exp(s+b) == exp(s)*exp(b), bias exp'd
                 on host (bf16) and DMA'd as contiguous 1MB per sp-pair
       mult   -> e_t = e_r * ebs, one DVE op per sp-PAIR (FD=4096, bf16 2x)
       PV     -> [V|1].T @ E accumulates O.T + softmax sums in PSUM f32;
                 groups queued >=4 deep so the PE always has ready work
     K1-3/Q1-3/V-b0 projections run as tasks popped under tb0 (V in NATURAL
     orientation: stationary = hsT chunk -> no VT/transpose phase; biases
     folded in via ones-row matmuls so drains are pure copies).
     b1's projections pop 1 task/sp under tb1-tb3, gated on their hsT DMAs.
     Block drains spread over the NEXT block as small tasks (no boundary
     bunching -> no PE idle window):
       O.T+sums -> SBUF bf16; PE-transpose puts tokens into PARTITIONS, so
       1/sums applies as a per-partition DVE scalar; PE-transpose back and the
       out-projection runs heads-FUSED (contraction 128) with ScalarE
       pure-copy drains.  No DRAM sums-roundtrip.
  C: attention b1 (same pipeline).  The LAST block's drain tasks round-robin
     across all three PSUM pools so the tail chains run in parallel.
Host: partial outputs summed over cores + bo (the all-reduce of out_proj).

PSUM budget (8 banks): st 2x2 + ot 2 + aux 2 (one rotating bank-sized tag
shared by fwd/back transposes, wo matmuls and task-popped proj groups).

Notes from failed experiments (do not retry blindly):
  - fp8e4m3 for E (probs) via DoubleRow PV: rel err 2.4e-2 (subnormal flush +
    6% mantissa steps on probs) AND slower (the gpsimd bf16->fp8 cast
    serialized the pipeline).
  - fp8e4m3 for Q/K/V projections (x16 weight scaling): rel err 4.5e-2 —
    score noise amplifies through exp ~5x worse than a naive error model.
  - PE runs mostly at the cold 1.2GHz HAM clock (MM avg ~370-390ns for N=512)
    despite ~80% busy; micro-gaps keep re-throttling it (K18-style).  PE is
    the binding engine at ~247us busy; ScalarE ~176, DVE ~145.
"""
import sys
from collections import deque

for _p in ("/opt/trn_rl_repo",):
    if _p not in sys.path:
        sys.path.append(_p)

import numpy as np

import concourse.bass as bass
import concourse.tile as tile
from concourse import mybir
from concourse.masks import make_identity
from concourse.bass_utils import run_bass_kernel_spmd

F32 = mybir.dt.float32
BF16 = mybir.dt.bfloat16
F8 = mybir.dt.float8e4

B, T, E, NH, D = 2, 2048, 1024, 16, 64
N_CORES = 8
HPC = NH // N_CORES          # 2 heads per core
DH = HPC * D                 # 128
SCALING = float(D * 2.0) ** -0.5
T_BLOCK = 512
NTB = T // T_BLOCK           # 4 t-blocks per batch
NSP = 8                      # sp-steps per block (2 s-tiles each)
NST = 16                     # s-tiles per batch
NE = E // 128                # 8 e-strips
TOK = B * T


def _waitfix(nc, limit=1):
    """This walrus build accepts at most ONE sync-wait per instruction.
    Hoist excess sem-waits onto inserted single-wait NoOps."""
    n_fixed = 0
    for bb in nc.m.functions[0].blocks:
        i = 0
        insts = bb.instructions
        while i < len(insts):
            inst = insts[i]
            si = inst.sync_info
            if si and si.on_wait and len(si.on_wait) > limit:
                extra = si.on_wait[limit:]
                si.on_wait = si.on_wait[:limit]
                for k, w in enumerate(extra):
                    nop = mybir.InstNoOp(
                        name=f"{inst.name}-waitfix{k}",
                        engine=inst.engine,
                        sync_info=mybir.SyncInfo(on_wait=[w], on_update=[]),
                        bass_nofuse=True,
                    )
                    nc.register_instruction(nop, overwrite=True)
                    insts.insert(i, nop)
                    i += 1
                n_fixed += 1
            i += 1
    return n_fixed


def build_attention_nc():
    nc = bass.Bass()

    hsT = nc.declare_dram_parameter("hsT", [E, TOK], BF16, isOutput=False)
    wqT = nc.declare_dram_parameter("wqT", [E, DH], BF16, isOutput=False)
    wkT = nc.declare_dram_parameter("wkT", [E, DH], BF16, isOutput=False)
    wvT = nc.declare_dram_parameter("wvT", [E, DH], BF16, isOutput=False)
    brows = nc.declare_dram_parameter("brows", [1, 3 * DH], BF16, isOutput=False)
    woT = nc.declare_dram_parameter("woT", [DH, E], BF16, isOutput=False)
    # exp(bias+mask), pre-arranged on host to per-(block, sp-pair) tiles:
    # [B, NTB, 4 pairs, 128 p, 2 spi, 2 a, 2 half, 512 t]
    bias_in = nc.declare_dram_parameter("bias",
                                        [B, NTB, 4, 128, 2, 2, 2, T_BLOCK],
                                        BF16, isOutput=False)
    out_partial = nc.declare_dram_parameter("out", [TOK, E], BF16, isOutput=True)

    with tile.TileContext(nc) as tc:
        from contextlib import ExitStack
        with ExitStack() as ctx:
            consts = ctx.enter_context(tc.tile_pool(name="consts", bufs=1))
            persist = ctx.enter_context(tc.tile_pool(name="persist", bufs=1))
            ebs_pool = ctx.enter_context(tc.tile_pool(name="ebs", bufs=4))
            er_pool = ctx.enter_context(tc.tile_pool(name="er", bufs=2))
            et_pool = ctx.enter_context(tc.tile_pool(name="et", bufs=2))
            hst_pool = ctx.enter_context(tc.tile_pool(name="hst", bufs=64))
            otn65_pool = ctx.enter_context(tc.tile_pool(name="otn65", bufs=2))
            onat_pool = ctx.enter_context(tc.tile_pool(name="onat", bufs=2))
            otnn_pool = ctx.enter_context(tc.tile_pool(name="otnn", bufs=2))
            rsb_pool = ctx.enter_context(tc.tile_pool(name="rsb", bufs=2))
            os_pool = ctx.enter_context(tc.tile_pool(name="osb", bufs=2))

            i_bf = consts.tile([128, 128], BF16, tag="i_bf")
            make_identity(nc, i_bf[:])
            ones_row = consts.tile([1, T_BLOCK], BF16, tag="ones_row")
            nc.gpsimd.memset(ones_row[:], 1.0)

            # dummy exp: pull the ACT exp table-load off the critical path
            dummy = consts.tile([128, 1], BF16, tag="dummy")
            nc.scalar.activation(out=dummy[:], in_=i_bf[:, 0:1],
                                 func=mybir.ActivationFunctionType.Exp)

            # weights: (E, DH) -> (128, NE, DH) bf16
            w_sb = {}
            for name, src in (("wq", wqT), ("wk", wkT), ("wv", wvT)):
                t = consts.tile([128, NE, DH], BF16, tag=name)
                nc.sync.dma_start(out=t[:],
                                  in_=src.rearrange("(n p) d -> p n d", p=128))
                w_sb[name] = t
            wo_sb = consts.tile([128, E], BF16, tag="wo")
            nc.sync.dma_start(out=wo_sb[:], in_=woT[:, :])
            b_rows = consts.tile([1, 3 * DH], BF16, tag="b_rows")
            nc.sync.dma_start(out=b_rows[:], in_=brows[:, :])

            # persistent activations
            QTb = [persist.tile([128, T], BF16, tag=f"QT{bb}", name=f"QT{bb}")
                   for bb in range(B)]
            KTb = [persist.tile([128, T], BF16, tag=f"KT{bb}", name=f"KT{bb}")
                   for bb in range(B)]
            V_sbb = []
            for bb in range(B):
                V_sb = persist.tile([128, NST, 256], BF16, tag=f"V_sb{bb}",
                                    name=f"V_sb{bb}")
                nc.vector.memset(V_sb[:, :, :], 0.0)
                nc.vector.memset(V_sb[:, :, D:D + 1], 1.0)
                nc.vector.memset(V_sb[:, :, 128 + D:128 + D + 1], 1.0)
                V_sbb.append(V_sb)

            # ---------- hsT strip DMA (per batch, chunk-major) ----------
            hstrips = {}

            def emit_hst_dma(bb, c, e):
                h = hst_pool.tile([128, T_BLOCK], BF16, tag="hst",
                                  name=f"hst{bb}_{c}_{e}")
                nc.sync.dma_start(
                    out=h[:], in_=hsT[e * 128:(e + 1) * 128,
                                      bb * T + c * T_BLOCK:
                                      bb * T + (c + 1) * T_BLOCK])
                hstrips[(bb, c, e)] = h

            # ---------- projection emitters (psum_pool passed in) ----------
            def emit_qk_group(pool, bb, name, dst, c):
                ps = pool.tile([128, T_BLOCK], F32, tag="aux",
                               name=f"pqk{bb}_{name}_{c}")
                for e in range(NE):
                    nc.tensor.matmul(ps[:], w_sb[name][:, e, :],
                                     hstrips[(bb, c, e)][:],
                                     start=(e == 0), stop=False)
                bi = {"wq": 0, "wk": 1}[name]
                nc.tensor.matmul(ps[:], b_rows[0:1, bi * DH:(bi + 1) * DH],
                                 ones_row[:], start=False, stop=True)
                nc.vector.tensor_copy(
                    out=dst[:, c * T_BLOCK:(c + 1) * T_BLOCK], in_=ps[:])

            def emit_v_group(pool, bb, st):
                # V natural: out [128 t, 128 dh] = sum_e hsT_chunk.T @ wvT
                c, q = st // 4, st % 4        # chunk, 128-col quarter
                ps = pool.tile([128, T_BLOCK], F32, tag="aux",
                               name=f"pv{bb}_{st}")
                for e in range(NE):
                    nc.tensor.matmul(
                        ps[:, 0:DH],
                        hstrips[(bb, c, e)][:, q * 128:(q + 1) * 128],
                        w_sb["wv"][:, e, :],
                        start=(e == 0), stop=False)
                nc.tensor.matmul(ps[:, 0:DH], ones_row[:, 0:128],
                                 b_rows[0:1, 2 * DH:3 * DH],
                                 start=False, stop=True)
                nc.vector.tensor_copy(out=V_sbb[bb][:, st, 0:D],
                                      in_=ps[:, 0:D])
                nc.vector.tensor_copy(out=V_sbb[bb][:, st, 128:128 + D],
                                      in_=ps[:, D:DH])

            # ---------- phase A: b0 K0/Q0 only (rest under tb0) ----------
            with tc.tile_pool(name="proj0", bufs=3, space="PSUM") as proj0:
                for e in range(NE):
                    emit_hst_dma(0, 0, e)
                emit_qk_group(proj0, 0, "wk", KTb[0], 0)
                emit_qk_group(proj0, 0, "wq", QTb[0], 0)

            # ---------- phases B & C: attention (+ b1 proj under B) ----------
            with tc.tile_pool(name="st_ps", bufs=2, space="PSUM") as st_ps, \
                 tc.tile_pool(name="ot_ps", bufs=2, space="PSUM") as ot_ps, \
                 tc.tile_pool(name="aux_ps", bufs=2, space="PSUM") as aux_ps:

                tasks = deque()       # small closures: drains of prev block
                proj_tasks = deque()  # b1 projection closures (phase B)
                pend_pv = deque()     # queued PV groups

                def emit_pv_group(group):
                    ots_p, bb_, e_t_t, spi_, sp_ = group
                    for a in range(HPC):
                        for half in range(2):
                            pst = sp_ * 2 + half
                            nc.tensor.matmul(
                                ots_p[a][:],
                                V_sbb[bb_][:, pst, a * 128:a * 128 + 128],
                                e_t_t[:, spi_, a, half, :],
                                start=(pst == 0), stop=(pst == NST - 1))

                def enqueue_block_tasks(b_, tb_, ots_, pools=None):
                    pools = pools or [aux_ps]
                    pcyc = [0]

                    def psum_tile(shape, dtype, nm):
                        pool = pools[pcyc[0] % len(pools)]
                        pcyc[0] += 1
                        tg = {id(aux_ps): "aux", id(st_ps): "st",
                              id(ot_ps): "ot"}[id(pool)]
                        return pool.tile(shape, dtype, tag=tg, name=nm)
                    tglob = b_ * T + tb_ * T_BLOCK
                    otn65 = otn65_pool.tile([128, HPC, T_BLOCK], BF16,
                                            tag="otn65", name=f"otn65_{b_}_{tb_}")
                    o_natn = onat_pool.tile([128, 8, D], BF16, tag="onat",
                                            name=f"onat_{b_}_{tb_}")
                    otn_n = otnn_pool.tile([128, T_BLOCK], BF16, tag="otnn",
                                           name=f"otnn_{b_}_{tb_}")
                    r_sb = rsb_pool.tile([128, 8], F32, tag="rsb",
                                         name=f"rsb_{b_}_{tb_}")
                    os_t = os_pool.tile([128, 4, E], BF16, tag="osb",
                                        name=f"osb_{b_}_{tb_}")

                    # otn65 copies emitted inline (all PV of this block already
                    # emitted) so later pool reuse sees the readers in order.
                    for a in range(HPC):
                        nc.vector.tensor_copy(out=otn65[0:D + 1, a, :],
                                              in_=ots_[a][0:D + 1, :])

                    def t_fwd(k, a):
                        idx = k * 2 + a
                        tr = psum_tile([128, D + 1], BF16,
                                       f"trf{b_}_{tb_}_{k}_{a}")
                        nc.tensor.transpose(
                            tr[:, :],
                            otn65[0:D + 1, a, k * 128:(k + 1) * 128],
                            i_bf[0:D + 1, 0:D + 1])
                        nc.vector.reciprocal(r_sb[:, idx:idx + 1],
                                             tr[:, D:D + 1])
                        nc.vector.tensor_scalar(
                            out=o_natn[:, idx, :], in0=tr[:, 0:D],
                            scalar1=r_sb[:, idx:idx + 1], scalar2=None,
                            op0=mybir.AluOpType.mult)

                    def t_back(k):
                        trb = psum_tile([128, 128], BF16,
                                        f"trb{b_}_{tb_}_{k}")
                        # [128 t, (2 a, 64 d)] -> [(a,d)=128, 128 t] in one shot
                        nc.tensor.transpose(trb[:, :],
                                            o_natn[:, k * 2:k * 2 + 2, :],
                                            i_bf[:, :])
                        nc.vector.tensor_copy(
                            out=otn_n[:, k * 128:(k + 1) * 128], in_=trb[:, :])

                    def t_wo(k, n0):
                        wp = psum_tile([128, 512], F32,
                                       f"wo{b_}_{tb_}_{k}_{n0}")
                        nc.tensor.matmul(wp[:], otn_n[:, k * 128:(k + 1) * 128],
                                         wo_sb[:, n0:n0 + 512],
                                         start=True, stop=True)
                        nc.scalar.activation(
                            out=os_t[:, k, n0:n0 + 512], in_=wp[:],
                            func=mybir.ActivationFunctionType.Copy)

                    def t_dma(k):
                        nc.gpsimd.dma_start(
                            out=out_partial[tglob + k * 128:
                                            tglob + (k + 1) * 128, :],
                            in_=os_t[:, k, :])

                    for k in range(4):
                        tasks.append(lambda k=k: t_fwd(k, 0))
                        tasks.append(lambda k=k: t_fwd(k, 1))
                        tasks.append(lambda k=k: t_back(k))
                        tasks.append(lambda k=k: t_wo(k, 0))
                        tasks.append(lambda k=k: t_wo(k, 512))
                        tasks.append(lambda k=k: t_dma(k))

                # b1 proj tasks (popped under b0 attention, phase B);
                # each entry = (#strips it needs emitted, closure)
                def enqueue_b1_proj():
                    for c in range(4):
                        need = (c + 1) * NE
                        proj_tasks.append((need,
                            lambda c=c: emit_qk_group(aux_ps, 1, "wk", KTb[1], c)))
                        proj_tasks.append((need,
                            lambda c=c: emit_qk_group(aux_ps, 1, "wq", QTb[1], c)))
                        for st in range(c * 4, c * 4 + 4):
                            proj_tasks.append((need,
                                lambda st=st: emit_v_group(aux_ps, 1, st)))
                enqueue_b1_proj()

                b0_tasks = deque()
                for c in range(1, 4):
                    b0_tasks.append(
                        lambda c=c: emit_qk_group(aux_ps, 0, "wk", KTb[0], c))
                for st in range(NST):
                    b0_tasks.append(lambda st=st: emit_v_group(aux_ps, 0, st))
                for c in range(1, 4):
                    b0_tasks.append(
                        lambda c=c: emit_qk_group(aux_ps, 0, "wq", QTb[0], c))

                nb1_strips = [0]

                def emit_b1_strips(n):
                    while nb1_strips[0] < 32 and n > 0:
                        i = nb1_strips[0]
                        emit_hst_dma(1, i // NE, i % NE)
                        nb1_strips[0] += 1
                        n -= 1

                ebs_tiles = {}

                def prefetch_ebs(b_, tb_, pair_):
                    if tb_ >= NTB:
                        b_, tb_ = b_ + 1, tb_ - NTB
                    if b_ >= B or (b_, tb_, pair_) in ebs_tiles:
                        return
                    t = ebs_pool.tile([128, 2, 2, 2, T_BLOCK], BF16, tag="ebs",
                                      name=f"ebs{b_}_{tb_}_{pair_}")
                    nc.sync.dma_start(out=t[:], in_=bias_in[b_, tb_, pair_])
                    ebs_tiles[(b_, tb_, pair_)] = t

                for pr in range(3):
                    prefetch_ebs(0, 0, pr)
                    for e in range(NE):
                        emit_hst_dma(0, pr + 1, e)
                prefetch_ebs(0, 0, 3)

                for b in range(B):
                    for tb in range(NTB):
                        ots = [ot_ps.tile([128, T_BLOCK], F32, tag="ot",
                                          name=f"ot{b}_{tb}_{a}")
                               for a in range(HPC)]
                        for pair in range(4):
                            prefetch_ebs(b, tb + (pair + 4) // 4,
                                         (pair + 4) % 4)
                            if b == 0:
                                emit_b1_strips(4)
                            ebs_t = ebs_tiles.pop((b, tb, pair))
                            e_r = er_pool.tile([128, 2, 2, 2, T_BLOCK], BF16,
                                               tag="er", name=f"er{b}_{tb}_{pair}")
                            e_t = et_pool.tile([128, 2, 2, 2, T_BLOCK], BF16,
                                               tag="et", name=f"et{b}_{tb}_{pair}")
                            for spi in range(2):
                                sp = pair * 2 + spi
                                for half in range(2):
                                    sti = sp * 2 + half
                                    st_t = st_ps.tile([128, 2, T_BLOCK], F32,
                                                      tag="st",
                                                      name=f"st{b}_{tb}_{sti}")
                                    for a in range(HPC):
                                        r0 = a * D
                                        for m0 in (0, 64):
                                            nc.tensor.matmul(
                                                st_t[m0:m0 + 64, a, :],
                                                KTb[b][r0:r0 + D,
                                                       sti * 128 + m0:
                                                       sti * 128 + m0 + 64],
                                                QTb[b][r0:r0 + D,
                                                       tb * T_BLOCK:
                                                       (tb + 1) * T_BLOCK],
                                                start=True, stop=True)
                                    nc.scalar.activation(
                                        out=e_r[:, spi, :, half, :],
                                        in_=st_t[:],
                                        func=mybir.ActivationFunctionType.Exp)
                                # per-sp mult: PV for this sp becomes
                                # ready one exp earlier
                                nc.vector.tensor_mul(out=e_t[:, spi],
                                                     in0=e_r[:, spi],
                                                     in1=ebs_t[:, spi])
                                pend_pv.append((ots, b, e_t, spi, sp))
                                while len(pend_pv) > 6:
                                    emit_pv_group(pend_pv.popleft())
                                # side work LAST: a stalled task op can only
                                # delay work that is already a period ahead
                                if b == 0:
                                    for _ in range(3):
                                        if b0_tasks:
                                            b0_tasks.popleft()()
                                if (b == 0 and tb >= 1 and proj_tasks
                                        and nb1_strips[0] >= proj_tasks[0][0]):
                                    proj_tasks.popleft()[1]()
                                for _ in range(4):
                                    if tasks:
                                        tasks.popleft()()
                        # flush this block's remaining PV groups before the
                        # drain tasks reference ots (emission-order = dep-order)
                        while pend_pv:
                            emit_pv_group(pend_pv.popleft())
                        last = (b == B - 1 and tb == NTB - 1)
                        enqueue_block_tasks(
                            b, tb, ots,
                            pools=[aux_ps, st_ps, ot_ps] if last else None)
                # epilogue
                while b0_tasks:
                    b0_tasks.popleft()()
                while proj_tasks:
                    proj_tasks.popleft()[1]()
                while tasks:
                    tasks.popleft()()
    _waitfix(nc)
    return nc


# ---------------- host-side prep ----------------

def shard_inputs(hidden_states, attn_bias, attention_mask, Wq, bq, Wk, bk, Wv, bv,
                 Wo, bo, c_attn, n_cores=8, scaling=None):
    """Build per-core input maps. Returns (in_maps, with_mask)."""
    import ml_dtypes
    bf16 = ml_dtypes.bfloat16
    Bb, Tt, Ee = hidden_states.shape
    NHh = c_attn.shape[0]
    Dd = Ee // NHh
    HPCc = NHh // n_cores
    DHh = HPCc * Dd

    with_mask = bool(np.any(attention_mask))
    hsT = np.ascontiguousarray(hidden_states.reshape(Bb * Tt, Ee).T).astype(bf16)
    bias4 = attn_bias.reshape(Bb, NHh, Tt, Tt)
    if with_mask:
        bias4 = bias4 + attention_mask.reshape(Bb, 1, Tt, Tt)

    if scaling is None:
        scaling = float(Dd * 2.0) ** -0.5

    expb_u16 = np.exp(bias4).astype(bf16).view(np.uint16)

    in_maps = []
    for c in range(n_cores):
        r0 = c * DHh
        sl = slice(r0, r0 + DHh)
        hsl = slice(c * HPCc, (c + 1) * HPCc)
        cvec = np.repeat(c_attn[c * HPCc:(c + 1) * HPCc], Dd)
        # ebs: [B, NTB, 4 pair, 128 p, 2 spi, 4 plane(2*half+a), 512 t]
        # value = exp(bias)[b, a, t, s] with s = (((pair*2+spi)*2+half)*128+p)
        eb = expb_u16[:, hsl]                       # [B, 2, T(t), S(s)]
        eb = eb.reshape(Bb, HPCc, NTB, T_BLOCK, 4, 2, 2, 128)
        # axes: b, a, tb, t', pair, spi, half, p -> b, tb, pair, p, spi, a, half, t'
        ebs = np.ascontiguousarray(eb.transpose(0, 2, 4, 7, 5, 1, 6, 3))
        brows = np.concatenate([(bq[sl] * scaling), bk[sl],
                                  (bv[sl] * cvec)])[None, :].astype(bf16)
        m = {
            "hsT": hsT,
            "wqT": np.ascontiguousarray((Wq[sl] * scaling).T).astype(bf16),
            "wkT": np.ascontiguousarray(Wk[sl].T).astype(bf16),
            "wvT": np.ascontiguousarray((Wv[sl] * cvec[:, None]).T).astype(bf16),
            "brows": brows,
            "woT": np.ascontiguousarray(Wo[:, sl].T).astype(bf16),
            "bias": ebs.view(bf16),
        }
        in_maps.append(m)
    return in_maps, with_mask


_NC_CACHE = {}


def run_spmd(in_maps, with_mask=False, **kwargs):
    if "nc" not in _NC_CACHE:
        _NC_CACHE["nc"] = build_attention_nc()
    nc = _NC_CACHE["nc"]
    return run_bass_kernel_spmd(nc, in_maps, list(range(N_CORES)), **kwargs)


def kernel(hidden_states, attn_bias, attention_mask, Wq, bq, Wk, bk, Wv, bv,
           Wo, bo, c_attn):
    args = [np.asarray(a, dtype=np.float32) for a in
            (hidden_states, attn_bias, attention_mask, Wq, bq, Wk, bk, Wv, bv,
             Wo, bo, c_attn)]
    (hidden_states, attn_bias, attention_mask, Wq, bq, Wk, bk, Wv, bv,
     Wo, bo, c_attn) = args
    in_maps, with_mask = shard_inputs(hidden_states, attn_bias, attention_mask,
                                      Wq, bq, Wk, bk, Wv, bv, Wo, bo, c_attn,
                                      n_cores=N_CORES, scaling=SCALING)
    res = run_spmd(in_maps, with_mask)
    out = np.zeros((B * T, E), np.float32)
    for r in res.results:
        out += r["out"]
    out += bo[None, :]
    return out.reshape(B, T, E).astype(np.float32)
